# revision 1
# baseline (speedup 1.0000x reference)
"""Trainium2 Bass kernel for the quirky MultiHeadAttention problem.

reference:
    scores = softmax(einsum('bhnd,bhmd->bhnm', q, k) * 8.0, axis=-1)
    out[b,h,m,d] = (sum_n scores[b,h,n,m]) * v[b,h,m,d]

q,k,v: [2, 16, 2048, 64] fp32.  32 (b,h) pairs sharded 4 per core across 8
NeuronCores (pure data parallelism).

Per (b,h) on-core (N=M=2048, D=64), per 128-row n-block:
  S   = (8*Q)block @ K^T          TensorE fp32r -> PSUM
  -mx = reduce_max(S, negate)     VectorE (only big DVE op in the loop)
  bias= min(-mx_a, -mx_b)         GpSimd
  P   = exp(S + bias) -> bf16     ScalarE, accum_out -> rowsums
  rscols[:, j] = rs_a + rs_b      GpSimd (delayed one block)
then per (b,h):
  wcols = 1/rscols                VectorE (one [128,16] reciprocal)
  c     = sum_j w_j^T @ P_j       TensorE bf16, [1, 1024] PSUM acc x2 halves,
                                  spread across the next bh's block loop
  c -> DRAM bounce -> [128, 16]   (SBUF partition-reshape DMA is not legal)
  out = c * v                     VectorE tensor_scalar per 128-col group
"""

from contextlib import ExitStack

import numpy as np

import concourse.tile as tile
import concourse.mybir as mybir
from concourse import bacc, bass_utils

F32 = mybir.dt.float32
F32R = mybir.dt.float32r
BF16 = mybir.dt.bfloat16
AX = mybir.AxisListType
AF = mybir.ActivationFunctionType
OP = mybir.AluOpType

B, H, N, D = 2, 16, 2048, 64
M = N
NCORES = 8
BH_PER_CORE = (B * H) // NCORES
SCALE = 8.0


def _build(n_bh=BH_PER_CORE, n=N, m=M, d=D, num_devices=NCORES, s_bufs=4,
           spread=2, finish_at=15, prefetch_at=0, bias_on_dve=False):
    m_half = 1024
    n_blocks = n // 128
    T = m // 128
    nc = bacc.Bacc("TRN2", target_bir_lowering=False, debug=False,
                   num_devices=num_devices)
    qt = nc.dram_tensor("qt", [n_bh, d, n], F32R, kind="ExternalInput").ap()
    kt = nc.dram_tensor("kt", [n_bh, d, m], F32R, kind="ExternalInput").ap()
    v = nc.dram_tensor("v", [n_bh, m, d], F32, kind="ExternalInput").ap()
    out = nc.dram_tensor("out", [n_bh, m, d], F32, kind="ExternalOutput").ap()

    with ExitStack() as ctx:
        tc = ctx.enter_context(tile.TileContext(nc))
        inp = ctx.enter_context(tc.tile_pool(name="inp", bufs=2))
        pp = ctx.enter_context(tc.tile_pool(name="pp", bufs=2 * n_blocks + 6))
        small = ctx.enter_context(tc.tile_pool(name="small", bufs=4))
        percol = ctx.enter_context(tc.tile_pool(name="percol", bufs=2))
        cb = ctx.enter_context(tc.tile_pool(name="cb", bufs=2))
        dscratch = ctx.enter_context(tc.tile_pool(name="dscratch", bufs=2,
                                                  space="DRAM"))
        sp = ctx.enter_context(tc.tile_pool(name="sp", bufs=s_bufs, space="PSUM"))

        st = {}

        def emit_dma_in(bh):
            qt_sb = inp.tile([d, n], F32R, tag="qt", name=f"qt{bh}")
            nc.sync.dma_start(qt_sb, qt[bh])
            kt_sb = inp.tile([d, m], F32R, tag="kt", name=f"kt{bh}")
            nc.sync.dma_start(kt_sb, kt[bh])
            v_sb = inp.tile([128, T * d], F32, tag="v", name=f"v{bh}")
            nc.sync.dma_start(v_sb, v[bh].rearrange("(p t) d -> p (t d)", p=128))
            st[bh] = dict(
                qt_sb=qt_sb, kt_sb=kt_sb, v_sb=v_sb,
                p_tiles=[[None, None] for _ in range(n_blocks)],
                rscols=percol.tile([128, n_blocks], F32, tag="rscols",
                                   name=f"rscols{bh}"),
                wcols=percol.tile([128, n_blocks], F32, tag="wcols",
                                  name=f"wcols{bh}"),
                wcols_bf=percol.tile([128, n_blocks], BF16, tag="wcols_bf",
                                     name=f"wcols_bf{bh}"),
                c_sb=None, pend_rs=None)

        def emit_block(bh, j):
            s = st[bh]
            lhsT = s["qt_sb"][:, j * 128:(j + 1) * 128]
            s_tiles, rms = [], []
            for h in range(2):
                s_t = sp.tile([128, m_half], F32, tag="S", name=f"s{bh}_{j}_{h}")
                for c in range(m_half // 512):
                    col0 = h * m_half + c * 512
                    nc.tensor.matmul(s_t[:, c * 512:(c + 1) * 512], lhsT,
                                     s["kt_sb"][:, col0:col0 + 512],
                                     start=True, stop=True)
                rm = small.tile([128, 1], F32, tag=f"rm{h}", name=f"rm{bh}_{j}_{h}")
                nc.vector.reduce_max(out=rm, in_=s_t, axis=AX.X, negate=True)
                s_tiles.append(s_t)
                rms.append(rm)
            bias_t = small.tile([128, 1], F32, tag="bias", name=f"bias{bh}_{j}")
            if bias_on_dve:
                nc.vector.tensor_scalar(out=bias_t, in0=rms[0], scalar1=rms[1],
                                        scalar2=None, op0=OP.min)
            else:
                nc.gpsimd.tensor_scalar(out=bias_t, in0=rms[0], scalar1=rms[1],
                                        scalar2=None, op0=OP.min)
            if s["pend_rs"] is not None:
                pj, r0, r1 = s["pend_rs"]
                nc.gpsimd.tensor_scalar(out=s["rscols"][:, pj:pj + 1], in0=r0,
                                        scalar1=r1, scalar2=None, op0=OP.add)
            rsx = []
            for h in range(2):
                p_t = pp.tile([128, m_half], BF16, tag="P", name=f"p{bh}_{j}_{h}")
                rs = small.tile([128, 1], F32, tag=f"rs{h}", name=f"rs{bh}_{j}_{h}")
                nc.scalar.activation(out=p_t, in_=s_tiles[h], func=AF.Exp,
                                     bias=bias_t, scale=1.0, accum_out=rs)
                s["p_tiles"][j][h] = p_t
                rsx.append(rs)
            s["pend_rs"] = (j, rsx[0], rsx[1])

        def emit_wfinal(bh):
            s = st[bh]
            pj, r0, r1 = s["pend_rs"]
            nc.gpsimd.tensor_scalar(out=s["rscols"][:, pj:pj + 1], in0=r0,
                                    scalar1=r1, scalar2=None, op0=OP.add)
            s["pend_rs"] = None
            nc.vector.reciprocal(out=s["wcols"], in_=s["rscols"])
            nc.gpsimd.tensor_copy(out=s["wcols_bf"], in_=s["wcols"])

        def emit_colsum_part(bh, h, j0, j1):
            s = st[bh]
            if s["c_sb"] is None:
                s["c_sb"] = cb.tile([1, m], F32, tag="c_sb", name=f"c_sb{bh}")
            if s.get(f"acc{h}") is None:
                s[f"acc{h}"] = sp.tile([1, m_half], F32, tag="S",
                                       name=f"acc{bh}_{h}")
            acc = s[f"acc{h}"]
            for j in range(j0, j1):
                for c in range(m_half // 512):
                    nc.tensor.matmul(acc[0:1, c * 512:(c + 1) * 512],
                                     s["wcols_bf"][:, j:j + 1],
                                     s["p_tiles"][j][h][:, c * 512:(c + 1) * 512],
                                     start=(j == 0), stop=(j == n_blocks - 1))
            if j1 == n_blocks:
                nc.vector.tensor_copy(
                    out=s["c_sb"][0:1, h * m_half:(h + 1) * m_half], in_=acc)

        def emit_colsum(bh, h):
            emit_colsum_part(bh, h, 0, n_blocks)

        def emit_finish(bh):
            s = st[bh]
            c_dram = dscratch.tile([1, m], F32, tag="c_dram", name=f"c_dram{bh}")
            nc.sync.dma_start(c_dram, s["c_sb"])
            c_cols = cb.tile([128, T], F32, tag="c_cols", name=f"c_cols{bh}")
            nc.sync.dma_start(c_cols, c_dram.rearrange("1 (p t) -> p t", p=128))
            out_sb = cb.tile([128, T * d], F32, tag="out_sb", name=f"out_sb{bh}")
            for t in range(T):
                nc.vector.tensor_scalar_mul(out_sb[:, t * d:(t + 1) * d],
                                            s["v_sb"][:, t * d:(t + 1) * d],
                                            c_cols[:, t:t + 1])
            nc.sync.dma_start(out[bh].rearrange("(p t) d -> p (t d)", p=128),
                              out_sb)
            s["p_tiles"] = None

        emit_dma_in(0)
        for bh in range(n_bh):
            for j in range(n_blocks):
                if j == prefetch_at and bh + 1 < n_bh:
                    emit_dma_in(bh + 1)
                emit_block(bh, j)
                if j == n_blocks - 1:
                    emit_wfinal(bh)
                if bh > 0 and st.get(bh - 1, {}).get("p_tiles") is not None:
                    # spread: `spread` j-chunks of pass A then pass B per block
                    total = 2 * n_blocks
                    done = min(j * spread, total)
                    todo = min((j + 1) * spread, total)
                    if done < n_blocks:
                        emit_colsum_part(bh - 1, 0, done, min(todo, n_blocks))
                    if todo > n_blocks and done < total:
                        emit_colsum_part(bh - 1, 1, max(done - n_blocks, 0),
                                         todo - n_blocks)
                    if j == finish_at:
                        emit_finish(bh - 1)
        emit_colsum(n_bh - 1, 0)
        emit_colsum(n_bh - 1, 1)
        emit_finish(n_bh - 1)
    nc.compile()
    return nc



_NC_CACHE = {}


def _get_nc():
    if "nc" not in _NC_CACHE:
        _NC_CACHE["nc"] = _build()
    return _NC_CACHE["nc"]


def _make_in_maps(q, k, v):
    q = np.asarray(q, dtype=np.float32).reshape(B * H, N, D)
    k = np.asarray(k, dtype=np.float32).reshape(B * H, M, D)
    v = np.asarray(v, dtype=np.float32).reshape(B * H, M, D)
    qs = (SCALE * q).transpose(0, 2, 1)            # [BH, D, N]
    kt = k.transpose(0, 2, 1)                      # [BH, D, M]
    in_maps = []
    for s_ in (slice(c * BH_PER_CORE, (c + 1) * BH_PER_CORE)
               for c in range(NCORES)):
        in_maps.append({
            "qt": np.ascontiguousarray(qs[s_]),
            "kt": np.ascontiguousarray(kt[s_]),
            "v": np.ascontiguousarray(v[s_]),
        })
    return in_maps


def _gather(results):
    parts = [results[core]["out"] for core in range(NCORES)]
    out = np.concatenate(parts, axis=0)  # [BH, M, D]
    return np.ascontiguousarray(out.reshape(B, H, M, D).astype(np.float32))


def kernel(q, k, v):
    nc = _get_nc()
    in_maps = _make_in_maps(q, k, v)
    res = bass_utils.run_bass_kernel_spmd(
        nc, in_maps, core_ids=list(range(NCORES)))
    return _gather(res.results)


def run_traced(inputs):
    """Run with NTFF profiling; returns exec_time_ns (or None)."""
    nc = _get_nc()
    in_maps = _make_in_maps(**inputs)
    res = bass_utils.run_bass_kernel_spmd(
        nc, in_maps, core_ids=list(range(NCORES)), trace=True)
    return res.exec_time_ns

